# revision 8
# baseline (speedup 1.0000x reference)
"""Positional-encoding add for Trainium2 (8 NeuronCores).

out[b, s, d] = x[b, s, d] + pe[s, d],  x: [8, 4096, 1024] f32.

Sharding: split the seq axis (4096) into 8 chunks of 512 — core c gets
x[:, c*512:(c+1)*512, :] (16 MiB) plus its 2 MiB pe slice, so per-core
HBM traffic is 34 MiB (vs 48 MiB for batch sharding, where the full
16 MiB pe table would be re-read by every core).

Device layout: the flat [8*512, 1024] shard is viewed as [1024, 4096].
512 consecutive flat rows are exactly one batch, so every [128, 4096]
tile of the view adds the SAME [128, 4096] view of the pe slice
(partition p of the view holds seq rows 4p..4p+3 in both x and pe).
pe loads into SBUF once; 8 2-MiB x tiles stream through tensor_add.
"""

import numpy as np

import concourse.bass as bass
import concourse.mybir as mybir
from concourse.bass_utils import run_bass_kernel_spmd

B, S, D = 8, 4096, 1024
NCORES = 8
S_SH = S // NCORES            # 512 seq positions per core
P = 128                       # SBUF partitions
W = 4096                      # free width of the device view
RV = (B * S_SH * D) // W      # 1024 device-view rows per core
NT = RV // P                  # 8 tiles per core

_CACHE = {}


def _positional_table() -> np.ndarray:
    # Bit-identical to the reference: same jnp (XLA CPU) fp32 ops.
    import jax
    import jax.numpy as jnp

    cpu = jax.devices("cpu")[0]
    with jax.default_device(cpu):
        pos = jnp.arange(S, dtype=jnp.float32)[:, None]
        even = jnp.arange(0, D, 2, dtype=jnp.float32) / D
        odd = jnp.arange(1, D, 2, dtype=jnp.float32) / D
        sin_part = jnp.sin(pos / jnp.power(10000.0, even))
        cos_part = jnp.cos(pos / jnp.power(10000.0, odd))
        pe = jnp.concatenate([sin_part, cos_part], axis=-1)[:, :D]
        return np.asarray(pe)


def _build_program():
    # Raw Bass (no TileContext): this container's walrus permits only ONE
    # embedded sync wait per instruction, which Tile's scheduler (and its
    # mandatory tail Drain) exceeds. Explicit wait_ge ops are standalone
    # single-sem instructions and compile fine.
    from contextlib import ExitStack

    nc = bass.Bass("TRN2")
    x = nc.declare_dram_parameter("x", [RV, W], mybir.dt.float32, isOutput=False)
    pe = nc.declare_dram_parameter("pe", [P, W], mybir.dt.float32, isOutput=False)
    out = nc.declare_dram_parameter("out", [RV, W], mybir.dt.float32, isOutput=True)

    with ExitStack() as st:
        pe_sb = st.enter_context(nc.sbuf_tensor("pe_sb", [P, W], mybir.dt.float32))
        tiles = [
            st.enter_context(nc.sbuf_tensor(f"t{i}", [P, W], mybir.dt.float32))
            for i in range(NT)
        ]
        pe_sem = st.enter_context(nc.semaphore("pe_sem"))
        x_sems = [st.enter_context(nc.semaphore(f"x_sem{i}")) for i in range(NT)]
        add_sem = st.enter_context(nc.semaphore("add_sem"))
        done_sem = st.enter_context(nc.semaphore("done_sem"))
        block = st.enter_context(nc.Block())

        @block.sync
        def _(sync):
            sync.dma_start(out=pe_sb[:], in_=pe[:]).then_inc(pe_sem, 16)
            for i in range(NT):
                sync.dma_start(
                    out=tiles[i][:], in_=x[i * P:(i + 1) * P, :]
                ).then_inc(x_sems[i], 16)

        @block.vector
        def _(vector):
            vector.wait_ge(pe_sem, 16)
            for i in range(NT):
                vector.wait_ge(x_sems[i], 16)
                nc.vector.tensor_add(
                    out=tiles[i][:], in0=tiles[i][:], in1=pe_sb[:]
                ).then_inc(add_sem, 1)

        @block.gpsimd
        def _(gpsimd):
            for i in range(NT):
                gpsimd.wait_ge(add_sem, i + 1)
                gpsimd.dma_start(
                    out=out[i * P:(i + 1) * P, :], in_=tiles[i][:]
                ).then_inc(done_sem, 16)
            gpsimd.wait_ge(done_sem, 16 * NT)
    return nc


def _get_program():
    if "nc" not in _CACHE:
        _CACHE["nc"] = _build_program()
        _CACHE["pe"] = _positional_table()
    return _CACHE["nc"], _CACHE["pe"]


def kernel(x: np.ndarray, _trace: bool = False):
    nc, pe = _get_program()
    x = np.asarray(x)
    in_maps = []
    for c in range(NCORES):
        xs = np.ascontiguousarray(x[:, c * S_SH:(c + 1) * S_SH, :]).reshape(RV, W)
        ps = np.ascontiguousarray(pe[c * S_SH:(c + 1) * S_SH, :]).reshape(P, W)
        in_maps.append({"x": xs, "pe": ps})
    res = run_bass_kernel_spmd(nc, in_maps, list(range(NCORES)), trace=_trace)
    out = np.empty((B, S, D), dtype=np.float32)
    for c in range(NCORES):
        out[:, c * S_SH:(c + 1) * S_SH, :] = res.results[c]["out"].reshape(B, S_SH, D)
    if _trace:
        return out, res
    return out


# revision 9
# speedup vs baseline: 1.0000x; 1.0000x over previous
"""Positional-encoding add for Trainium2 (8 NeuronCores).

out[b, s, d] = x[b, s, d] + pe[s, d],  x: [8, 4096, 1024] f32.

Sharding: split the seq axis (4096) into 8 chunks of 512 — core c gets
x[:, c*512:(c+1)*512, :] (16 MiB) plus its 2 MiB pe slice, so per-core
HBM traffic is 34 MiB (vs 48 MiB for batch sharding, where the full
16 MiB pe table would be re-read by every core).

Device layout: the flat [8*512, 1024] shard is viewed as [1024, 4096].
512 consecutive flat rows are exactly one batch, so every [128, 4096]
tile of the view adds the SAME [128, 4096] view of the pe slice
(partition p of the view holds seq rows 4p..4p+3 in both x and pe).
pe loads into SBUF once; 8 2-MiB x tiles stream through tensor_add.
"""

import numpy as np

import concourse.bass as bass
import concourse.mybir as mybir
from concourse.bass_utils import run_bass_kernel_spmd

B, S, D = 8, 4096, 1024
NCORES = 8
S_SH = S // NCORES            # 512 seq positions per core
P = 128                       # SBUF partitions
W = 4096                      # free width of the device view
RV = (B * S_SH * D) // W      # 1024 device-view rows per core
NT = RV // P                  # 8 tiles per core

_CACHE = {}


def _positional_table() -> np.ndarray:
    # Bit-identical to the reference: same jnp (XLA CPU) fp32 ops.
    import jax
    import jax.numpy as jnp

    cpu = jax.devices("cpu")[0]
    with jax.default_device(cpu):
        pos = jnp.arange(S, dtype=jnp.float32)[:, None]
        even = jnp.arange(0, D, 2, dtype=jnp.float32) / D
        odd = jnp.arange(1, D, 2, dtype=jnp.float32) / D
        sin_part = jnp.sin(pos / jnp.power(10000.0, even))
        cos_part = jnp.cos(pos / jnp.power(10000.0, odd))
        pe = jnp.concatenate([sin_part, cos_part], axis=-1)[:, :D]
        return np.asarray(pe)


def _build_program():
    # Raw Bass (no TileContext): this container's walrus permits only ONE
    # embedded sync wait per instruction, which Tile's scheduler (and its
    # mandatory tail Drain) exceeds. Explicit wait_ge ops are standalone
    # single-sem instructions and compile fine.
    from contextlib import ExitStack

    nc = bass.Bass("TRN2")
    x = nc.declare_dram_parameter("x", [RV, W], mybir.dt.float32, isOutput=False)
    pe = nc.declare_dram_parameter("pe", [P, W], mybir.dt.float32, isOutput=False)
    out = nc.declare_dram_parameter("out", [RV, W], mybir.dt.float32, isOutput=True)

    with ExitStack() as st:
        pe_sb = st.enter_context(nc.sbuf_tensor("pe_sb", [P, W], mybir.dt.float32))
        tiles = [
            st.enter_context(nc.sbuf_tensor(f"t{i}", [P, W], mybir.dt.float32))
            for i in range(NT)
        ]
        pe_sem = st.enter_context(nc.semaphore("pe_sem"))
        x_sems = [st.enter_context(nc.semaphore(f"x_sem{i}")) for i in range(NT)]
        add_sem = st.enter_context(nc.semaphore("add_sem"))
        done_sem = st.enter_context(nc.semaphore("done_sem"))
        block = st.enter_context(nc.Block())

        @block.sync
        def _(sync):
            # pe split into NT column chunks so the one-time 2 MiB table
            # load spreads across all DMA queues instead of doubling one
            # queue's traffic. All chunks bump one sem: single-wait consume.
            pc = W // NT
            for j in range(NT):
                sync.dma_start(
                    out=pe_sb[:, j * pc:(j + 1) * pc],
                    in_=pe[:, j * pc:(j + 1) * pc],
                ).then_inc(pe_sem, 16)
            for i in range(NT):
                sync.dma_start(
                    out=tiles[i][:], in_=x[i * P:(i + 1) * P, :]
                ).then_inc(x_sems[i], 16)

        @block.vector
        def _(vector):
            vector.wait_ge(pe_sem, 16 * NT)
            for i in range(NT):
                vector.wait_ge(x_sems[i], 16)
                nc.vector.tensor_add(
                    out=tiles[i][:], in0=tiles[i][:], in1=pe_sb[:]
                ).then_inc(add_sem, 1)

        @block.gpsimd
        def _(gpsimd):
            for i in range(NT):
                gpsimd.wait_ge(add_sem, i + 1)
                gpsimd.dma_start(
                    out=out[i * P:(i + 1) * P, :], in_=tiles[i][:]
                ).then_inc(done_sem, 16)
            gpsimd.wait_ge(done_sem, 16 * NT)
    return nc


def _get_program():
    if "nc" not in _CACHE:
        _CACHE["nc"] = _build_program()
        _CACHE["pe"] = _positional_table()
    return _CACHE["nc"], _CACHE["pe"]


def kernel(x: np.ndarray, _trace: bool = False):
    nc, pe = _get_program()
    x = np.asarray(x)
    in_maps = []
    for c in range(NCORES):
        xs = np.ascontiguousarray(x[:, c * S_SH:(c + 1) * S_SH, :]).reshape(RV, W)
        ps = np.ascontiguousarray(pe[c * S_SH:(c + 1) * S_SH, :]).reshape(P, W)
        in_maps.append({"x": xs, "pe": ps})
    res = run_bass_kernel_spmd(nc, in_maps, list(range(NCORES)), trace=_trace)
    out = np.empty((B, S, D), dtype=np.float32)
    for c in range(NCORES):
        out[:, c * S_SH:(c + 1) * S_SH, :] = res.results[c]["out"].reshape(B, S_SH, D)
    if _trace:
        return out, res
    return out
